# revision 1
# baseline (speedup 1.0000x reference)
"""GCN model (3x GCNConv + 2x BatchNorm + global mean pool + linear) on 8 TRN2 cores.

Strategy:
- Host: add self-loops as explicit edges; bin-pack nodes into 392 bins of <=128
  slots balancing per-bin edge counts; remap node ids to (bin, slot); assign 49
  bins per core (edges partitioned by dst bin); pad each bin's edge list to T
  tiles of 128 edges.
- Device (SPMD x8): GCNConv aggregation = indirect-DMA gather of 128 bf16
  feature rows + one-hot (iota compare) matmul accumulating into a PSUM tile
  per 128-dst-slot bin.  deg^-1/2 norms folded in as row pre-scale (on the
  gathered table) and per-partition post-scale.  conv bias added inside PSUM
  via a K=1 matmul of (1/dis) x b.  BatchNorms folded into the following
  matmul's weights (scale) + a rank-1 PSUM correction (shift).  Mean-pool via
  one-hot matmul.  Cross-core: bf16 AllGather of the node-feature table
  between layers, small AllReduduce for BN stats / pooled sums.
"""

import os

import numpy as np
import ml_dtypes

N = 50000
E = 800000
IN = 128
HID = 256
G = 64
NCLS = 10
BN_EPS = 1e-5

P = 128
CORES = 8
BINS = 392          # global 128-slot bins (392*128 = 50176 slots)
CH = BINS // CORES  # 49 bins per core
SL = CH * P         # 6272 slots per core
S = BINS * P        # 50176 total slots

F32 = np.float32
BF16 = ml_dtypes.bfloat16

LAST_EXEC_NS = None
LAST_RESULTS = None


def _preprocess(x, edge_index, batch):
    import heapq

    src = edge_index[0].astype(np.int64)
    dst = edge_index[1].astype(np.int64)
    deg = 1.0 + np.bincount(dst, minlength=N).astype(np.float64)
    dis = (1.0 / np.sqrt(deg)).astype(F32)
    invdis = np.sqrt(deg).astype(F32)

    # ---- bin-pack nodes into BINS bins (<=128 nodes each), balancing edges ----
    w = np.bincount(dst, minlength=N).astype(np.int64) + 1  # incl. self-loop
    order = np.argsort(-w, kind="stable")
    heap = [(0, b) for b in range(BINS)]
    heapq.heapify(heap)
    count = np.zeros(BINS, np.int64)
    new_id = np.empty(N, np.int64)
    for n in order:
        while True:
            load, b = heapq.heappop(heap)
            if count[b] < P:
                break
        new_id[n] = b * P + count[b]
        count[b] += 1
        heapq.heappush(heap, (load + int(w[n]), b))

    # ---- edges (with self-loops), grouped by dst bin, padded to tiles ----
    es = np.concatenate([src, np.arange(N, dtype=np.int64)])
    ed = np.concatenate([dst, np.arange(N, dtype=np.int64)])
    es_s = new_id[es]
    ed_s = new_id[ed]
    bin_e = ed_s // P
    o = np.argsort(bin_e, kind="stable")
    es_s, ed_s, bin_e = es_s[o], ed_s[o], bin_e[o]
    cnt_bin = np.bincount(bin_e, minlength=BINS)
    T = int(np.ceil(cnt_bin.max() / P))
    cap = T * P
    starts = np.concatenate([[0], np.cumsum(cnt_bin)[:-1]])
    rank = np.arange(len(es_s)) - starts[bin_e]
    pos = bin_e * cap + rank
    src_pad = np.zeros(BINS * cap, np.int32)
    dst_pad = np.full(BINS * cap, 255.0, F32)
    src_pad[pos] = es_s.astype(np.int32)
    dst_pad[pos] = (ed_s % P).astype(F32)
    src_tiles = src_pad.reshape(BINS * T, P)  # [tiles, 128]
    dst_tiles = dst_pad.reshape(BINS * T, P)

    # ---- per-slot arrays ----
    slot_dis = np.zeros(S, F32)
    slot_dis[new_id] = dis
    slot_invdis = np.zeros(S, F32)
    slot_invdis[new_id] = invdis
    slot_batch = np.full(S, 255.0, F32)
    slot_batch[new_id] = batch.astype(F32)
    xT = np.zeros((IN, S), F32)
    xT[:, new_id] = x.T
    xT = xT.astype(BF16)

    cnts = np.bincount(batch.astype(np.int64), minlength=G).astype(F32)
    cnts = np.maximum(cnts, 1.0)

    per_core = []
    for c in range(CORES):
        t0, t1 = c * CH * T, (c + 1) * CH * T
        s0, s1 = c * SL, (c + 1) * SL
        per_core.append(
            dict(
                srcidx=np.ascontiguousarray(src_tiles[t0:t1].T),       # [128, CH*T] i32
                dstloc=np.ascontiguousarray(dst_tiles[t0:t1].T),       # [128, CH*T] f32
                disloc=np.ascontiguousarray(slot_dis[s0:s1].reshape(CH, P).T),   # [128, CH]
                invdis=np.ascontiguousarray(slot_invdis[s0:s1].reshape(1, SL)),  # [1, SL]
                batchloc=np.ascontiguousarray(slot_batch[s0:s1].reshape(CH, P).T),  # [128, CH]
            )
        )

    shared = dict(
        xT=xT,
        disall=np.ascontiguousarray(slot_dis.reshape(BINS, P).T),  # [128, BINS]
        iota128=np.tile(np.arange(P, dtype=F32), (P, 1)),          # [128,128]
        eye128=np.eye(P, dtype=F32),
        onescol=np.ones((P, 1), F32),
        onesrow=np.ones((1, P), F32),
        cntrow=cnts.reshape(1, G),
        invcntcol=(1.0 / cnts).reshape(G, 1),
    )
    return per_core, shared, T


def _build(nc, tc, T):
    from concourse import bass, mybir
    STOP = os.environ.get('K_STOP', '')

    f32 = mybir.dt.float32
    bf16 = mybir.dt.bfloat16
    i32 = mybir.dt.int32
    AF = mybir.ActivationFunctionType
    OP = mybir.AluOpType
    NT = CH * T  # tiles per core per layer

    # ---------------- parameters ----------------
    def par(name, shape, dt):
        return nc.declare_dram_parameter(name, list(shape), dt, isOutput=False)

    xT_d = par("xT", (IN, S), bf16)
    srcidx_d = par("srcidx", (P, NT), i32)
    dstloc_d = par("dstloc", (P, NT), f32)
    disloc_d = par("disloc", (P, CH), f32)
    invdis_d = par("invdis", (1, SL), f32)
    batch_d = par("batchloc", (P, CH), f32)
    disall_d = par("disall", (P, BINS), f32)
    iota_d = par("iota128", (P, P), f32)
    eye_d = par("eye128", (P, P), f32)
    onescol_d = par("onescol", (P, 1), f32)
    onesrow_d = par("onesrow", (1, P), f32)
    cntrow_d = par("cntrow", (1, G), f32)
    invcnt_d = par("invcntcol", (G, 1), f32)
    W1_d = par("W1", (IN, HID), f32)
    W2_d = par("W2", (HID, HID), f32)
    W3_d = par("W3", (HID, HID), f32)
    Wf_d = par("Wf", (HID, NCLS), f32)
    b1_d = par("b1", (1, HID), f32)
    b2_d = par("b2", (1, HID), f32)
    b3_d = par("b3", (1, HID), f32)
    bf_d = par("bf", (1, NCLS), f32)
    g1_d = par("g1c", (P, 2), f32)
    be1_d = par("be1c", (P, 2), f32)
    g2_d = par("g2c", (P, 2), f32)
    be2_d = par("be2c", (P, 2), f32)
    out_d = nc.declare_dram_parameter("out", [G, NCLS], f32, isOutput=True)

    # ---------------- device DRAM ----------------
    hs1_d = nc.dram_tensor("hs1", [S, HID], bf16)
    hs2_d = nc.dram_tensor("hs2", [S, HID], bf16)
    hs3_d = nc.dram_tensor("hs3", [S, HID], bf16)
    hsloc2_d = nc.dram_tensor("hsloc2", [SL, HID], bf16)
    hsloc3_d = nc.dram_tensor("hsloc3", [SL, HID], bf16)
    st1_in = nc.dram_tensor("st1_in", [P, 4], f32)
    st1_out = nc.dram_tensor("st1_out", [P, 4], f32)
    ar2_in = nc.dram_tensor("ar2_in", [P, 132], f32)
    ar2_out = nc.dram_tensor("ar2_out", [P, 132], f32)

    GRP = [list(range(CORES))]

    # ---------------- resident SBUF ----------------
    import contextlib

    ctx = contextlib.ExitStack()
    res = ctx.enter_context(tc.tile_pool(name="res", bufs=1))
    psr = ctx.enter_context(tc.tile_pool(name="psr", bufs=1, space="PSUM"))

    hloc = res.tile([P, CH * HID], f32)       # resident node features [slot, feat]
    hT0 = res.tile([P, SL], bf16)             # transposed features, feat block 0
    hT1 = res.tile([P, SL], bf16)
    srcidx = res.tile([P, NT], i32)
    dstloc = res.tile([P, NT], f32)
    disloc = res.tile([P, CH], f32)
    invdis = res.tile([1, SL], f32)
    batchloc = res.tile([P, CH], f32)
    disall = res.tile([P, BINS], f32)
    iota = res.tile([P, P], f32)
    eye = res.tile([P, P], f32)
    onescol = res.tile([P, 1], f32)
    onesrow = res.tile([1, P], f32)
    cntrow = res.tile([1, G], f32)
    invcnt = res.tile([G, 1], f32)
    W1 = res.tile([IN, HID], bf16)
    W2s = [res.tile([P, HID], bf16, tag=f"w2_{f}", name=f"w2_{f}") for f in range(2)]
    W3s = [res.tile([P, HID], f32, tag=f"w3_{f}", name=f"w3_{f}") for f in range(2)]
    W3p = [res.tile([P, HID], bf16, tag=f"w3p_{f}", name=f"w3p_{f}") for f in range(2)]
    Wfs = [res.tile([P, NCLS], f32, tag=f"wf_{f}", name=f"wf_{f}") for f in range(2)]
    Wfp = [res.tile([P, NCLS], f32, tag=f"wfp_{f}", name=f"wfp_{f}") for f in range(2)]
    b1 = res.tile([1, HID], f32)
    b2 = res.tile([1, HID], f32)
    b3 = res.tile([1, HID], f32)
    bfr = res.tile([1, NCLS], f32)
    g1c = res.tile([P, 2], f32)
    be1c = res.tile([P, 2], f32)
    g2c = res.tile([P, 2], f32)
    be2c = res.tile([P, 2], f32)
    rrow = res.tile([1, HID], f32)
    bfp = res.tile([1, NCLS], f32)
    scale1 = res.tile([P, 2], f32)
    shift1 = res.tile([P, 2], f32)
    scale2 = res.tile([P, 2], f32)
    shift2 = res.tile([P, 2], f32)
    stats1 = res.tile([P, 4], f32)
    ar2 = res.tile([P, 132], f32)
    epscol = res.tile([P, 1], f32)
    nc.vector.memset(epscol[:], BN_EPS)

    dma = nc.sync.dma_start
    for dst_t, src_t in [
        (srcidx, srcidx_d), (dstloc, dstloc_d), (disloc, disloc_d),
        (invdis, invdis_d), (batchloc, batch_d), (disall, disall_d),
        (iota, iota_d), (eye, eye_d), (onescol, onescol_d),
        (onesrow, onesrow_d), (cntrow, cntrow_d), (invcnt, invcnt_d),
        (b1, b1_d), (b2, b2_d), (b3, b3_d), (bfr, bf_d),
        (g1c, g1_d), (be1c, be1_d), (g2c, g2_d), (be2c, be2_d),
    ]:
        dma(out=dst_t[:], in_=src_t[:, :])
    # weights: cast f32 -> bf16 through SBUF
    wtmp_pool = tc.alloc_tile_pool(name="wtmp", bufs=2)
    wt = wtmp_pool.tile([IN, HID], f32, tag="wt")
    dma(out=wt[:], in_=W1_d[:, :])
    nc.vector.tensor_copy(out=W1[:], in_=wt[:])
    for f in range(2):
        wt2 = wtmp_pool.tile([P, HID], f32, tag="wt")
        dma(out=wt2[:], in_=W2_d[f * P:(f + 1) * P, :])
        nc.vector.tensor_copy(out=W2s[f][:], in_=wt2[:])
        dma(out=W3s[f][:], in_=W3_d[f * P:(f + 1) * P, :])
        dma(out=Wfs[f][:], in_=Wf_d[f * P:(f + 1) * P, :])
    wtmp_pool.release()


    def _early_out(tag):
        with tc.tile_pool(name="eo_" + tag, bufs=1) as eo:
            z = eo.tile([G, NCLS], f32, tag="z", name="z_" + tag)
            nc.vector.tensor_copy(out=z[:], in_=hloc[0:G, 0:NCLS])
            dma(out=out_d[:, :], in_=z[:])
    # ================= phase A: hs1 = dis * (x @ W1), all slots =================
    with tc.tile_pool(name="pA", bufs=4) as pA, \
         tc.tile_pool(name="pAp", bufs=2, space="PSUM") as pAp:
        for g in range(BINS):
            xt = pA.tile([P, P], bf16, tag="xt")
            dma(out=xt[:], in_=xT_d[:, g * P:(g + 1) * P])
            ps = pAp.tile([P, HID], f32, tag="ps")
            nc.tensor.matmul(out=ps[:], lhsT=xt[:], rhs=W1[:], start=True, stop=True)
            hs = pA.tile([P, HID], bf16, tag="hs")
            nc.scalar.activation(out=hs[:], in_=ps[:], func=AF.Copy,
                                 scale=disall[:, g:g + 1])
            dma(out=hs1_d[g * P:(g + 1) * P, :], in_=hs[:])

    if STOP == 'A':
        _early_out('A'); ctx.close(); return

    # ================= edge aggregation pass =================
    def edge_pass(hs_table, bias_row, out_getter):
        """out_getter(j) -> SBUF AP [128, HID] destination for relu'd result."""
        with tc.tile_pool(name="pE", bufs=6) as pE, \
             tc.tile_pool(name="pEp", bufs=2, space="PSUM") as pEp:
            for j in range(CH):
                ps = pEp.tile([P, HID], f32, tag="agg")
                for t in range(T):
                    ti = j * T + t
                    gt = pE.tile([P, HID], bf16, tag="gath")
                    nc.gpsimd.indirect_dma_start(
                        out=gt[:], out_offset=None,
                        in_=hs_table[:, :],
                        in_offset=bass.IndirectOffsetOnAxis(
                            ap=srcidx[:, ti:ti + 1], axis=0),
                    )
                    oh = pE.tile([P, P], bf16, tag="oh")
                    nc.vector.tensor_tensor(
                        out=oh[:], in0=dstloc[:, ti:ti + 1].to_broadcast([P, P]),
                        in1=iota[:], op=OP.is_equal)
                    nc.tensor.matmul(out=ps[:], lhsT=oh[:], rhs=gt[:],
                                     start=(t == 0), stop=False)
                # + (1/dis) x bias  (K=1 rank-1 update), then relu(dis * psum)
                nc.tensor.matmul(out=ps[:], lhsT=invdis[0:1, j * P:(j + 1) * P],
                                 rhs=bias_row[:], start=False, stop=True)
                nc.scalar.activation(out=out_getter(j), in_=ps[:], func=AF.Relu,
                                     scale=disloc[:, j:j + 1])

    def hchunk(j):
        return hloc[:, j * HID:(j + 1) * HID]

    edge_pass(hs1_d, b1, hchunk)
    if STOP == 'E1':
        _early_out('E1'); ctx.close(); return

    # ============== transpose hloc -> hT (bf16) ==============
    def transpose_h():
        with tc.tile_pool(name="pT", bufs=4) as pT, \
             tc.tile_pool(name="pTp", bufs=4, space="PSUM") as pTp:
            for j in range(CH):
                for f, hT in enumerate((hT0, hT1)):
                    pst = pTp.tile([P, P], f32, tag="pst")
                    nc.tensor.transpose(
                        out=pst[:], in_=hloc[:, j * HID + f * P: j * HID + (f + 1) * P],
                        identity=eye[:])
                    nc.scalar.activation(out=hT[:, j * P:(j + 1) * P], in_=pst[:],
                                         func=AF.Copy)

    # ============== hs_next = dis * (h @ W) [+ dis x r], allgather ==============
    def make_hs(Ws, hsloc_dram, hs_dram, add_r):
        with tc.tile_pool(name="pH", bufs=4) as pH, \
             tc.tile_pool(name="pHp", bufs=2, space="PSUM") as pHp:
            for j in range(CH):
                ps = pHp.tile([P, HID], f32, tag="hs")
                nc.tensor.matmul(out=ps[:], lhsT=hT0[:, j * P:(j + 1) * P],
                                 rhs=Ws[0][:], start=True, stop=False)
                nc.tensor.matmul(out=ps[:], lhsT=hT1[:, j * P:(j + 1) * P],
                                 rhs=Ws[1][:], start=False, stop=not add_r)
                if add_r:
                    nc.tensor.matmul(out=ps[:], lhsT=onesrow[:],
                                     rhs=rrow[:], start=False, stop=True)
                hst = pH.tile([P, HID], bf16, tag="hst")
                nc.scalar.activation(out=hst[:], in_=ps[:], func=AF.Copy,
                                     scale=disloc[:, j:j + 1])
                dma(out=hsloc_dram[j * P:(j + 1) * P, :], in_=hst[:])
        nc.gpsimd.collective_compute(
            "AllGather", mybir.AluOpType.bypass, replica_groups=GRP,
            ins=[hsloc_dram.ap().opt()], outs=[hs_dram.ap().opt()])

    transpose_h()
    if STOP == 'T1':
        _early_out('T1'); ctx.close(); return
    make_hs(W2s, hsloc2_d, hs2_d, add_r=False)
    if STOP == 'H2':
        _early_out('H2'); ctx.close(); return
    edge_pass(hs2_d, b2, hchunk)
    if STOP == 'E2':
        _early_out('E2'); ctx.close(); return

    # ============== BN1 stats -> allreduce -> scale1/shift1 ==============
    with tc.tile_pool(name="pS", bufs=4) as pS, \
         tc.tile_pool(name="pSp", bufs=1, space="PSUM") as pSp:
        s1 = [pSp.tile([P, 1], f32, tag=f"s1_{f}", name=f"s1_{f}") for f in range(2)]
        s2 = [pSp.tile([P, 1], f32, tag=f"s2_{f}", name=f"s2_{f}") for f in range(2)]
        for j in range(CH):
            sq = pS.tile([P, HID], f32, tag="sq")
            nc.scalar.activation(out=sq[:], in_=hchunk(j), func=AF.Square)
            for f in range(2):
                hsl = hloc[:, j * HID + f * P: j * HID + (f + 1) * P]
                nc.tensor.matmul(out=s1[f][:], lhsT=hsl, rhs=onescol[:],
                                 start=(j == 0), stop=(j == CH - 1))
                nc.tensor.matmul(out=s2[f][:], lhsT=sq[:, f * P:(f + 1) * P],
                                 rhs=onescol[:], start=(j == 0), stop=(j == CH - 1))
        st = pS.tile([P, 4], f32, tag="st")
        for f in range(2):
            nc.vector.tensor_copy(out=st[:, f:f + 1], in_=s1[f][:])
            nc.vector.tensor_copy(out=st[:, 2 + f:3 + f], in_=s2[f][:])
        dma(out=st1_in[:, :], in_=st[:])
    nc.gpsimd.collective_compute(
        "AllReduce", mybir.AluOpType.add, replica_groups=GRP,
        ins=[st1_in.ap().opt()], outs=[st1_out.ap().opt()])
    dma(out=stats1[:], in_=st1_out[:, :])

    def bn_fold(stats_sums, stats_sqs, gc, bec, scale_t, shift_t, pool):
        """stats cols -> scale/shift [P,2] (feature-column layout)."""
        mu = pool.tile([P, 2], f32, tag="mu")
        var = pool.tile([P, 2], f32, tag="var")
        tmp = pool.tile([P, 2], f32, tag="tmp")
        nc.vector.tensor_scalar_mul(out=mu[:], in0=stats_sums, scalar1=1.0 / N)
        nc.vector.tensor_scalar_mul(out=var[:], in0=stats_sqs, scalar1=1.0 / N)
        nc.vector.tensor_tensor(out=tmp[:], in0=mu[:], in1=mu[:], op=OP.mult)
        nc.vector.tensor_tensor(out=var[:], in0=var[:], in1=tmp[:], op=OP.subtract)
        nc.scalar.activation(out=tmp[:], in_=var[:], func=AF.Sqrt, bias=epscol[:])
        nc.vector.reciprocal(out=tmp[:], in_=tmp[:])
        nc.vector.tensor_tensor(out=scale_t[:], in0=gc[:], in1=tmp[:], op=OP.mult)
        nc.vector.tensor_tensor(out=tmp[:], in0=mu[:], in1=scale_t[:], op=OP.mult)
        nc.vector.tensor_tensor(out=shift_t[:], in0=bec[:], in1=tmp[:], op=OP.subtract)

    with tc.tile_pool(name="pB", bufs=1) as pB, \
         tc.tile_pool(name="pBp", bufs=1, space="PSUM") as pBp:
        bn_fold(stats1[:, 0:2], stats1[:, 2:4], g1c, be1c, scale1, shift1, pB)
        # W3' = scale1 (*) W3 rows; rrow = shift1 @ W3
        psr_ = pBp.tile([1, HID], f32, tag="rr")
        for f in range(2):
            w3f = pB.tile([P, HID], f32, tag="w3f")
            nc.vector.tensor_scalar_mul(out=w3f[:], in0=W3s[f][:],
                                        scalar1=scale1[:, f:f + 1])
            nc.vector.tensor_copy(out=W3p[f][:], in_=w3f[:])
            nc.tensor.matmul(out=psr_[:], lhsT=shift1[:, f:f + 1], rhs=W3s[f][:],
                             start=(f == 0), stop=(f == 1))
        nc.vector.tensor_copy(out=rrow[:], in_=psr_[:])

    if STOP == 'B1':
        _early_out('B1'); ctx.close(); return
    transpose_h()
    make_hs(W3p, hsloc3_d, hs3_d, add_r=True)
    if STOP == 'H3':
        _early_out('H3'); ctx.close(); return
    edge_pass(hs3_d, b3, hchunk)
    if STOP == 'E3':
        _early_out('E3'); ctx.close(); return

    # ====== BN2 stats + pooled sums -> one allreduce ======
    with tc.tile_pool(name="pG", bufs=4) as pG, \
         tc.tile_pool(name="pGp", bufs=1, space="PSUM") as pGp:
        pool_ps = [pGp.tile([P, G], f32, tag=f"pool_{f}", name=f"pool_{f}") for f in range(2)]
        s1 = [pGp.tile([P, 1], f32, tag=f"gs1_{f}", name=f"gs1_{f}") for f in range(2)]
        s2 = [pGp.tile([P, 1], f32, tag=f"gs2_{f}", name=f"gs2_{f}") for f in range(2)]
        for j in range(CH):
            sq = pG.tile([P, HID], f32, tag="sq")
            nc.scalar.activation(out=sq[:], in_=hchunk(j), func=AF.Square)
            ohp = pG.tile([P, G], f32, tag="ohp")
            nc.vector.tensor_tensor(
                out=ohp[:], in0=batchloc[:, j:j + 1].to_broadcast([P, G]),
                in1=iota[:, 0:G], op=OP.is_equal)
            for f in range(2):
                hsl = hloc[:, j * HID + f * P: j * HID + (f + 1) * P]
                nc.tensor.matmul(out=pool_ps[f][:], lhsT=hsl, rhs=ohp[:],
                                 start=(j == 0), stop=(j == CH - 1))
                nc.tensor.matmul(out=s1[f][:], lhsT=hsl, rhs=onescol[:],
                                 start=(j == 0), stop=(j == CH - 1))
                nc.tensor.matmul(out=s2[f][:], lhsT=sq[:, f * P:(f + 1) * P],
                                 rhs=onescol[:], start=(j == 0), stop=(j == CH - 1))
        arp = pG.tile([P, 132], f32, tag="arp")
        for f in range(2):
            nc.vector.tensor_copy(out=arp[:, f * G:(f + 1) * G], in_=pool_ps[f][:])
            nc.vector.tensor_copy(out=arp[:, 128 + f:129 + f], in_=s1[f][:])
            nc.vector.tensor_copy(out=arp[:, 130 + f:131 + f], in_=s2[f][:])
        dma(out=ar2_in[:, :], in_=arp[:])
    nc.gpsimd.collective_compute(
        "AllReduce", mybir.AluOpType.add, replica_groups=GRP,
        ins=[ar2_in.ap().opt()], outs=[ar2_out.ap().opt()])
    dma(out=ar2[:], in_=ar2_out[:, :])

    # ====== fold BN2 into Wf, final matmul ======
    with tc.tile_pool(name="pF", bufs=1) as pF, \
         tc.tile_pool(name="pFp", bufs=1, space="PSUM") as pFp:
        bn_fold(ar2[:, 128:130], ar2[:, 130:132], g2c, be2c, scale2, shift2, pF)
        psb = pFp.tile([1, NCLS], f32, tag="psb")
        for f in range(2):
            nc.vector.tensor_scalar_mul(out=Wfp[f][:], in0=Wfs[f][:],
                                        scalar1=scale2[:, f:f + 1])
            nc.tensor.matmul(out=psb[:], lhsT=shift2[:, f:f + 1], rhs=Wfs[f][:],
                             start=(f == 0), stop=False)
        nc.tensor.matmul(out=psb[:], lhsT=onesrow[0:1, 0:1], rhs=bfr[:],
                         start=False, stop=True)
        nc.vector.tensor_copy(out=bfp[:], in_=psb[:])

        pso = pFp.tile([G, NCLS], f32, tag="pso")
        for f in range(2):
            nc.tensor.matmul(out=pso[:], lhsT=ar2[:, f * G:(f + 1) * G],
                             rhs=Wfp[f][:], start=(f == 0), stop=False)
        nc.tensor.matmul(out=pso[:], lhsT=cntrow[:], rhs=bfp[:],
                         start=False, stop=True)
        osb = pF.tile([G, NCLS], f32, tag="osb")
        nc.vector.tensor_scalar_mul(out=osb[:], in0=pso[:], scalar1=invcnt[:])
        dma(out=out_d[:, :], in_=osb[:])

    ctx.close()


def kernel(x, edge_index, batch, W1, b1, W2, b2, W3, b3, g1, be1, g2, be2, Wf, bf):
    global LAST_EXEC_NS, LAST_RESULTS
    from concourse import bacc, tile
    from concourse.bass_utils import run_bass_kernel_spmd

    x = np.asarray(x)
    edge_index = np.asarray(edge_index)
    batch = np.asarray(batch)

    per_core, shared, T = _preprocess(x, edge_index, batch)

    nc = bacc.Bacc("TRN2", target_bir_lowering=False, debug=False,
                   num_devices=CORES)
    with tile.TileContext(nc) as tc:
        _build(nc, tc, T)
    nc.compile()

    def col2(v):
        return np.ascontiguousarray(np.asarray(v, F32).reshape(2, P).T)

    base = dict(
        xT=shared["xT"], disall=shared["disall"].astype(F32),
        iota128=shared["iota128"], eye128=shared["eye128"],
        onescol=shared["onescol"], onesrow=shared["onesrow"],
        cntrow=shared["cntrow"], invcntcol=shared["invcntcol"],
        W1=np.asarray(W1, F32), W2=np.asarray(W2, F32), W3=np.asarray(W3, F32),
        Wf=np.asarray(Wf, F32),
        b1=np.asarray(b1, F32).reshape(1, HID), b2=np.asarray(b2, F32).reshape(1, HID),
        b3=np.asarray(b3, F32).reshape(1, HID), bf=np.asarray(bf, F32).reshape(1, NCLS),
        g1c=col2(g1), be1c=col2(be1), g2c=col2(g2), be2c=col2(be2),
    )
    in_maps = []
    for c in range(CORES):
        m = dict(base)
        m.update(per_core[c])
        in_maps.append(m)

    res = run_bass_kernel_spmd(nc, in_maps, core_ids=list(range(CORES)))
    LAST_EXEC_NS = res.exec_time_ns
    LAST_RESULTS = res
    return np.asarray(res.results[0]["out"], F32)



# revision 9
# speedup vs baseline: 1.4567x; 1.4567x over previous
"""GCN model (3x GCNConv + 2x BatchNorm + global mean pool + linear) on 8 TRN2 cores.

v3 strategy (vs v1 baseline):
- Host: bin-pack nodes into 392 bins of <=128 slots balancing per-bin edge
  counts (self-loops removed); remap node ids to (bin, slot); 49 bins per core
  (edges partitioned by dst bin).  Edges of each bin are split by src-table
  half (dma_gather indices are int16), padded per half to T_half tiles of 128.
  Upload a dis-prescaled x table [S, 128] bf16 as the layer-1 gather table
  (A(XW1) == (AX)W1, so W1 is applied after aggregation).
- Device (SPMD x8): batched dma_gather (SWDGE) fetches all source rows for a
  GROUP of bins in one instruction per table half (amortizes the ~1us fixed
  SWDGE cost that dominated v1's per-128-row indirect DMAs).  Per dst bin:
  one-hot (iota vs per-edge dst scalar, bf16 tensor_scalar 4x DVE mode)
  matmuls accumulate into PSUM; self-loop contribution via an identity matmul
  on the resident local table block (no gather); conv bias via K=1 bf16 matmul
  of invdis x b; relu + deg^-1/2 post-scale on the PSUM->SBUF copy.  Layer 1
  aggregates in [feat, dst] form so agg@W1 needs no transpose; layers 2/3 in
  [dst, feat] form with per-bin PE transposes into hT for the next layer's
  table matmuls.  Next-layer table rows stay resident in SBUF (reused for
  self-loops) and are stored via 7-bin batched DMAs.
- BatchNorms folded into following matmul weights (scale) + rank-1 shift.
- Cross-core: bf16 AllGather of node-feature tables (Shared scratchpad
  outputs) between layers; small AllReduces for BN stats / pooled sums.
"""

import os

import numpy as np
import ml_dtypes

N = 50000
E = 800000
IN = 128
HID = 256
G = 64
NCLS = 10
BN_EPS = 1e-5

P = 128
CORES = 8
BINS = 392          # global 128-slot bins (392*128 = 50176 slots)
CH = BINS // CORES  # 49 bins per core
SL = CH * P         # 6272 slots per core
S = BINS * P        # 50176 total slots
SHALF = S // 2      # dma_gather idx is int16: split table into two halves

F32 = np.float32
BF16 = ml_dtypes.bfloat16
I16 = np.int16

LAST_EXEC_NS = None
LAST_RESULTS = None


def _preprocess(x, edge_index, batch):
    import heapq

    src = edge_index[0].astype(np.int64)
    dst = edge_index[1].astype(np.int64)
    deg = 1.0 + np.bincount(dst, minlength=N).astype(np.float64)
    dis = (1.0 / np.sqrt(deg)).astype(F32)
    invdis = np.sqrt(deg).astype(F32)

    # ---- bin-pack nodes into BINS bins (<=128 nodes each), balancing edges ----
    w = np.bincount(dst, minlength=N).astype(np.int64)  # no self-loops
    order = np.argsort(-w, kind="stable")
    heap = [(0, b) for b in range(BINS)]
    heapq.heapify(heap)
    count = np.zeros(BINS, np.int64)
    new_id = np.empty(N, np.int64)
    for n in order:
        while True:
            load, b = heapq.heappop(heap)
            if count[b] < P:
                break
        new_id[n] = b * P + count[b]
        count[b] += 1
        heapq.heappush(heap, (load + int(w[n]), b))

    # ---- edges (NO self-loops), grouped by (dst bin, src half) ----
    es_s = new_id[src]
    ed_s = new_id[dst]
    bin_e = ed_s // P
    half_e = (es_s >= SHALF).astype(np.int64)
    key = bin_e * 2 + half_e
    o = np.argsort(key, kind="stable")
    es_s, ed_s, key = es_s[o], ed_s[o], key[o]
    half_s = key % 2
    cnt = np.bincount(key, minlength=BINS * 2)
    T_half = int(np.ceil(cnt.max() / P))
    cap = T_half * P
    starts = np.concatenate([[0], np.cumsum(cnt)[:-1]])
    rank = np.arange(len(es_s)) - starts[key]
    pos = key * cap + rank
    idx_pad = np.zeros(BINS * 2 * cap, np.int64)
    dst_pad = np.full(BINS * 2 * cap, 255.0, F32)
    idx_pad[pos] = es_s - half_s * SHALF          # row index within half table
    assert idx_pad.min() >= 0 and idx_pad.max() < SHALF
    dst_pad[pos] = (ed_s % P).astype(F32)
    # [bin, half, T_half, P]
    idx_pad = idx_pad.reshape(BINS, 2, T_half, P)
    dst_pad = dst_pad.reshape(BINS, 2, T_half, P)
    TT = 2 * T_half

    # ---- per-slot arrays ----
    slot_dis = np.zeros(S, F32)
    slot_dis[new_id] = dis
    slot_invdis = np.zeros(S, F32)
    slot_invdis[new_id] = invdis
    slot_batch = np.full(S, 255.0, F32)
    slot_batch[new_id] = batch.astype(F32)

    # layer-1 gather table: dis-prescaled x rows, [S, IN] bf16
    xs = np.zeros((S, IN), F32)
    xs[new_id, :] = x * dis[:, None]
    xs = xs.astype(BF16)
    xs_resh = xs.reshape(BINS, P, IN)  # [bin, slot, feat]

    cnts = np.bincount(batch.astype(np.int64), minlength=G).astype(F32)
    cnts = np.maximum(cnts, 1.0)

    def wrap16(lin):
        """Linear int array -> [128, len/16] int16, wrapped in 16 partitions."""
        w = lin.reshape(-1, 16).T.astype(I16)     # [16, len/16]
        return np.ascontiguousarray(np.tile(w, (8, 1)))

    per_core = []
    for c in range(CORES):
        b0, b1 = c * CH, (c + 1) * CH
        s0, s1 = c * SL, (c + 1) * SL
        xsl = np.transpose(xs_resh[b0:b1], (1, 0, 2))  # [P, CH, IN]
        # dstloc: [128, CH*TT]; bin-major, lo tiles then hi tiles
        dstl = dst_pad[b0:b1].reshape(CH * TT, P).T
        # gather index streams (bin-major within each half), linear order
        # (tile, lane):  [CH, T_half, P] -> flat
        idxlo = idx_pad[b0:b1, 0].reshape(-1)
        idxhi = idx_pad[b0:b1, 1].reshape(-1)
        per_core.append(
            dict(
                idxlo16=wrap16(idxlo),                                 # [128, CH*T_half*8] i16
                idxhi16=wrap16(idxhi),
                dstloc=np.ascontiguousarray(dstl),                      # [128, CH*TT] f32
                disloc=np.ascontiguousarray(slot_dis[s0:s1].reshape(CH, P).T),   # [128, CH]
                invdis=np.ascontiguousarray(
                    slot_invdis[s0:s1].reshape(1, SL)).astype(BF16),   # [1, SL]
                batchloc=np.ascontiguousarray(
                    slot_batch[s0:s1].reshape(CH, P).T).astype(BF16),  # [128, CH]
                xsloc=np.ascontiguousarray(xsl.reshape(P, CH * IN)),   # [128, CH*IN] bf16
            )
        )

    iota = np.tile(np.arange(P, dtype=F32), (P, 1))
    shared = dict(
        xs=xs,                                                     # [S, IN] bf16
        iota128=iota.astype(BF16),                                 # [128,128]
        eye128=np.eye(P, dtype=F32).astype(BF16),                  # [128,128]
        onescolb=np.ones((P, 1), F32).astype(BF16),
        onesrowb=np.ones((1, P), F32).astype(BF16),
        ones11=np.ones((1, 1), F32),
        cntrow=cnts.reshape(1, G),
        invcntcol=(1.0 / cnts).reshape(G, 1),
    )
    return per_core, shared, T_half


def _build(nc, tc, T_half):
    from concourse import bass, mybir
    STOP = os.environ.get('K_STOP', '')
    SHARED = os.environ.get('K_LOCAL_TABLES', '') != '1'

    f32 = mybir.dt.float32
    bf16 = mybir.dt.bfloat16
    i16 = mybir.dt.int16
    AF = mybir.ActivationFunctionType
    OP = mybir.AluOpType
    TT = 2 * T_half
    NI16 = CH * T_half * 8   # idx16 cols per half

    # ---------------- parameters ----------------
    def par(name, shape, dt):
        return nc.declare_dram_parameter(name, list(shape), dt, isOutput=False)

    xs_d = par("xs", (S, IN), bf16)
    idxlo_d = par("idxlo16", (P, NI16), i16)
    idxhi_d = par("idxhi16", (P, NI16), i16)
    dstloc_d = par("dstloc", (P, CH * TT), f32)
    disloc_d = par("disloc", (P, CH), f32)
    invdis_d = par("invdis", (1, SL), bf16)
    batch_d = par("batchloc", (P, CH), bf16)
    xsloc_d = par("xsloc", (P, CH * IN), bf16)
    iota_d = par("iota128", (P, P), bf16)
    eye_d = par("eye128", (P, P), bf16)
    onescolb_d = par("onescolb", (P, 1), bf16)
    onesrowb_d = par("onesrowb", (1, P), bf16)
    ones11_d = par("ones11", (1, 1), f32)
    cntrow_d = par("cntrow", (1, G), f32)
    invcnt_d = par("invcntcol", (G, 1), f32)
    W1_d = par("W1", (IN, HID), bf16)
    W2_d = par("W2", (HID, HID), bf16)
    W3_d = par("W3", (HID, HID), f32)
    Wf_d = par("Wf", (HID, NCLS), f32)
    b1_d = par("b1", (1, HID), bf16)
    b2_d = par("b2", (1, HID), bf16)
    b3_d = par("b3", (1, HID), bf16)
    bf_d = par("bf", (1, NCLS), f32)
    g1_d = par("g1c", (P, 2), f32)
    be1_d = par("be1c", (P, 2), f32)
    g2_d = par("g2c", (P, 2), f32)
    be2_d = par("be2c", (P, 2), f32)
    out_d = nc.declare_dram_parameter("out", [G, NCLS], f32, isOutput=True)

    # ---------------- device DRAM ----------------
    addr = dict(addr_space="Shared") if SHARED else {}
    hsloc2_d = nc.dram_tensor("hsloc2", [SL, HID], bf16)
    hsloc3_d = nc.dram_tensor("hsloc3", [SL, HID], bf16)
    hs2_d = nc.dram_tensor("hs2", [S, HID], bf16, **addr)
    hs3_d = nc.dram_tensor("hs3", [S, HID], bf16, **addr)
    st1_in = nc.dram_tensor("st1_in", [P, 4], f32)
    st1_out = nc.dram_tensor("st1_out", [P, 4], f32)
    ar2_in = nc.dram_tensor("ar2_in", [P, 132], f32)
    ar2_out = nc.dram_tensor("ar2_out", [P, 132], f32)

    GRP = [list(range(CORES))]

    # ---------------- resident SBUF ----------------
    import contextlib

    ctx = contextlib.ExitStack()
    res = ctx.enter_context(tc.tile_pool(name="res", bufs=1))

    idxlo = res.tile([P, NI16], i16)
    idxhi = res.tile([P, NI16], i16)
    dstloc = res.tile([P, CH * TT], f32)
    disloc = res.tile([P, CH], f32)
    invdis = res.tile([1, SL], bf16)
    batchloc = res.tile([P, CH], bf16)
    xsloc = res.tile([P, CH * IN], bf16)
    hloc = res.tile([P, CH * HID], bf16)    # conv output h per local bin
    hT0 = res.tile([P, SL], bf16)           # h transposed, feat block 0
    hT1 = res.tile([P, SL], bf16)
    hsloc = res.tile([P, CH * HID], bf16)   # next-layer table rows (local bins)
    iota = res.tile([P, P], bf16)
    eye = res.tile([P, P], bf16)
    onescolb = res.tile([P, 1], bf16)
    onesrowb = res.tile([1, P], bf16)
    ones11 = res.tile([1, 1], f32)
    cntrow = res.tile([1, G], f32)
    invcnt = res.tile([G, 1], f32)
    W1 = res.tile([IN, HID], bf16)
    W2s = [res.tile([P, HID], bf16, tag=f"w2_{f}", name=f"w2_{f}") for f in range(2)]
    W3s = [res.tile([P, HID], f32, tag=f"w3_{f}", name=f"w3_{f}") for f in range(2)]
    W3p = [res.tile([P, HID], bf16, tag=f"w3p_{f}", name=f"w3p_{f}") for f in range(2)]
    Wfs = [res.tile([P, NCLS], f32, tag=f"wf_{f}", name=f"wf_{f}") for f in range(2)]
    Wfp = [res.tile([P, NCLS], f32, tag=f"wfp_{f}", name=f"wfp_{f}") for f in range(2)]
    b1 = res.tile([1, HID], bf16)
    b2 = res.tile([1, HID], bf16)
    b3 = res.tile([1, HID], bf16)
    bfr = res.tile([1, NCLS], f32)
    g1c = res.tile([P, 2], f32)
    be1c = res.tile([P, 2], f32)
    g2c = res.tile([P, 2], f32)
    be2c = res.tile([P, 2], f32)
    rrow = res.tile([1, HID], f32)
    rrowb = res.tile([1, HID], bf16)
    bfp = res.tile([1, NCLS], f32)
    scale1 = res.tile([P, 2], f32)
    shift1 = res.tile([P, 2], f32)
    scale2 = res.tile([P, 2], f32)
    shift2 = res.tile([P, 2], f32)
    stats1 = res.tile([P, 4], f32)
    ar2 = res.tile([P, 132], f32)
    epscol = res.tile([P, 1], f32)
    nc.vector.memset(epscol[:], BN_EPS)

    dma = nc.sync.dma_start
    for dst_t, src_t in [
        (idxlo, idxlo_d), (idxhi, idxhi_d), (dstloc, dstloc_d),
        (disloc, disloc_d), (invdis, invdis_d), (batchloc, batch_d),
        (xsloc, xsloc_d), (iota, iota_d), (eye, eye_d),
        (onescolb, onescolb_d), (onesrowb, onesrowb_d), (ones11, ones11_d),
        (cntrow, cntrow_d), (invcnt, invcnt_d), (W1, W1_d),
        (b1, b1_d), (b2, b2_d), (b3, b3_d), (bfr, bf_d),
        (g1c, g1_d), (be1c, be1_d), (g2c, g2_d), (be2c, be2_d),
    ]:
        dma(out=dst_t[:], in_=src_t[:, :])
    for f in range(2):
        dma(out=W2s[f][:], in_=W2_d[f * P:(f + 1) * P, :])
        dma(out=W3s[f][:], in_=W3_d[f * P:(f + 1) * P, :])
        dma(out=Wfs[f][:], in_=Wf_d[f * P:(f + 1) * P, :])

    # ---------------- AP helpers ----------------
    def ap3(tile2d, col0, k, inner):
        """View tile2d[:, col0:col0+k*inner] as [128, k, inner]."""
        a = tile2d[:, col0:col0 + k * inner]
        return bass.AP(a.tensor, a.offset, [list(a.ap[0]), [inner, k], [1, inner]])

    def store_rows(dram_t, j0, nb, sb_tile):
        """DMA sb_tile[:, j0*HID:(j0+nb)*HID] -> dram rows [j0*128,(j0+nb)*128)."""
        full = dram_t.ap()
        d_ap = bass.AP(full.tensor, j0 * P * HID,
                       [[HID, P], [P * HID, nb], [1, HID]])
        s = sb_tile[:, j0 * HID:(j0 + nb) * HID]
        s_ap = bass.AP(s.tensor, s.offset, [list(s.ap[0]), [HID, nb], [1, HID]])
        dma(out=d_ap, in_=s_ap)

    def hchunk(j):
        return hloc[:, j * HID:(j + 1) * HID]

    CT = 8                                 # tiles per dma_gather (ring limit)
    NSTREAM = CH * T_half                  # tiles per half-stream per pass

    def make_stream(pool, table, elem, tag):
        """Streamed dma_gather over a pass's tiles: one call per CT tiles per
        half.  get(half, pos) -> SBUF AP of that tile; ensure(j) prefetches
        through the end of bin j."""
        tiles = {0: {}, 1: {}}
        halves = ((idxlo, table[0:SHALF, :]), (idxhi, table[SHALF:S, :]))

        def _issue(h, c):
            if c in tiles[h] or c * CT >= NSTREAM:
                return
            n_t = min(CT, NSTREAM - c * CT)
            idx_t, rows = halves[h]
            gt = pool.tile([P, CT * elem], bf16, tag=f"{tag}{h}")
            n = n_t * P
            nc.gpsimd.dma_gather(
                ap3(gt, 0, n_t, elem), rows,
                idx_t[:, c * CT * 8:c * CT * 8 + n // 16],
                n, n, elem,
            )
            tiles[h][c] = gt

        def ensure(j):
            cmax = min((j + 1) * T_half + T_half - 1, NSTREAM - 1) // CT
            for h in (0, 1):
                c0 = max(tiles[h].keys(), default=-1) + 1
                for c in range(c0, cmax + 1):
                    _issue(h, c)

        def get(h, pos):
            c, s = divmod(pos, CT)
            return tiles[h][c][:, s * elem:(s + 1) * elem]

        return ensure, get

    def onehot(pool, j, t, tag):
        """One-hot [edge, dst] tile for bin j, tile t (of TT), bf16."""
        oh = pool.tile([P, P], bf16, tag=tag)
        nc.vector.tensor_scalar(
            out=oh[:], in0=iota[:], scalar1=dstloc[:, j * TT + t:j * TT + t + 1],
            scalar2=None, op0=OP.is_equal)
        return oh

    def transpose_to_hT(pT, j):
        for f, hT in enumerate((hT0, hT1)):
            pt = pT.tile([P, P], bf16, tag=f"pt{f}")
            nc.tensor.transpose(
                out=pt[:], in_=hloc[:, j * HID + f * P: j * HID + (f + 1) * P],
                identity=eye[:])
            nc.vector.tensor_copy(out=hT[:, j * P:(j + 1) * P], in_=pt[:])

    def _early_out(tag):
        with tc.tile_pool(name="eo_" + tag, bufs=1) as eo:
            z = eo.tile([G, NCLS], f32, tag="z", name="z_" + tag)
            nc.vector.tensor_copy(out=z[:], in_=hloc[0:G, 0:NCLS])
            dma(out=out_d[:, :], in_=z[:])

    # ============ E1: layer-1 aggregation from xs + h1 + hs2 rows ============
    with tc.tile_pool(name="pG1", bufs=4) as pG1, \
         tc.tile_pool(name="pOh1", bufs=4) as pOh1, \
         tc.tile_pool(name="pA1", bufs=2) as pA1, \
         tc.tile_pool(name="pPs1", bufs=2, space="PSUM") as pPs1, \
         tc.tile_pool(name="pH1", bufs=2, space="PSUM") as pH1, \
         tc.tile_pool(name="pT1", bufs=1, space="PSUM") as pT1:
        ens1, get1 = make_stream(pG1, xs_d, IN, "g1")
        for j in range(CH):
            ens1(j)
            if True:
                # aggT[feat, dst] in PSUM: self-loop (identity) + TT edge tiles
                ps1 = pPs1.tile([P, P], f32, tag="agg")
                nc.tensor.matmul(out=ps1[:], lhsT=xsloc[:, j * IN:(j + 1) * IN],
                                 rhs=eye[:], start=True, stop=False)
                for t in range(TT):
                    h = 0 if t < T_half else 1
                    pos = j * T_half + (t if t < T_half else t - T_half)
                    oh = onehot(pOh1, j, t, "oh")
                    nc.tensor.matmul(out=ps1[:], lhsT=get1(h, pos),
                                     rhs=oh[:], start=False, stop=(t == TT - 1))
                aT = pA1.tile([P, P], bf16, tag="aT")
                nc.vector.tensor_copy(out=aT[:], in_=ps1[:])
                # h1 = relu(dis * (agg @ W1) + b1)
                ps2 = pH1.tile([P, HID], f32, tag="h1")
                nc.tensor.matmul(out=ps2[:], lhsT=aT[:], rhs=W1[:],
                                 start=True, stop=False)
                nc.tensor.matmul(out=ps2[:], lhsT=invdis[0:1, j * P:(j + 1) * P],
                                 rhs=b1[:], start=False, stop=True)
                nc.scalar.activation(out=hchunk(j), in_=ps2[:], func=AF.Relu,
                                     scale=disloc[:, j:j + 1])
                transpose_to_hT(pT1, j)
                # hs2 row = dis * (h1 @ W2)
                ps3 = pH1.tile([P, HID], f32, tag="hs2")
                nc.tensor.matmul(out=ps3[:], lhsT=hT0[:, j * P:(j + 1) * P],
                                 rhs=W2s[0][:], start=True, stop=False)
                nc.tensor.matmul(out=ps3[:], lhsT=hT1[:, j * P:(j + 1) * P],
                                 rhs=W2s[1][:], start=False, stop=True)
                nc.scalar.activation(out=hsloc[:, j * HID:(j + 1) * HID], in_=ps3[:],
                                     func=AF.Copy, scale=disloc[:, j:j + 1])
                if j % 7 == 6:
                    store_rows(hsloc2_d, j - 6, 7, hsloc)
    nc.gpsimd.collective_compute(
        "AllGather", mybir.AluOpType.bypass, replica_groups=GRP,
        ins=[hsloc2_d.ap().opt()], outs=[hs2_d.ap().opt()])

    if STOP == 'E1':
        _early_out('E1'); ctx.close(); return

    # ============ E2/E3: layers 2/3 aggregation from hs tables ============
    def edge_pass(hs_table, bias_row, do_hT, pfx):
        with tc.tile_pool(name=pfx + "G", bufs=4) as pG, \
             tc.tile_pool(name=pfx + "Oh", bufs=4) as pOh, \
             tc.tile_pool(name=pfx + "Ps", bufs=2, space="PSUM") as pPs, \
             tc.tile_pool(name=pfx + "T", bufs=2, space="PSUM") as pT:
            ens, get = make_stream(pG, hs_table, HID, "g")
            for j in range(CH):
                ens(j)
                if True:
                    ps = pPs.tile([P, HID], f32, tag="agg")
                    for t in range(TT):
                        h = 0 if t < T_half else 1
                        pos = j * T_half + (t if t < T_half else t - T_half)
                        oh = onehot(pOh, j, t, "oh")
                        nc.tensor.matmul(out=ps[:], lhsT=oh[:],
                                         rhs=get(h, pos),
                                         start=(t == 0), stop=False)
                    # self-loop: psum[dst,:] += hsloc_bin[dst,:]
                    nc.tensor.matmul(out=ps[:], lhsT=eye[:],
                                     rhs=hsloc[:, j * HID:(j + 1) * HID],
                                     start=False, stop=False)
                    # + (1/dis) x bias (K=1), then relu(dis * psum)
                    nc.tensor.matmul(out=ps[:], lhsT=invdis[0:1, j * P:(j + 1) * P],
                                     rhs=bias_row[:], start=False, stop=True)
                    nc.scalar.activation(out=hchunk(j), in_=ps[:], func=AF.Relu,
                                         scale=disloc[:, j:j + 1])
                    if do_hT:
                        transpose_to_hT(pT, j)

    edge_pass(hs2_d, b2, do_hT=True, pfx="p2")
    if STOP == 'E2':
        _early_out('E2'); ctx.close(); return

    # ============== BN1 stats -> allreduce -> scale1/shift1 ==============
    with tc.tile_pool(name="pS", bufs=4) as pS, \
         tc.tile_pool(name="pSp", bufs=1, space="PSUM") as pSp:
        s1 = [pSp.tile([P, 1], f32, tag=f"s1_{f}", name=f"s1_{f}") for f in range(2)]
        s2 = [pSp.tile([P, 1], f32, tag=f"s2_{f}", name=f"s2_{f}") for f in range(2)]
        for j in range(CH):
            sq = pS.tile([P, HID], bf16, tag="sq")
            nc.scalar.activation(out=sq[:], in_=hchunk(j), func=AF.Square)
            for f in range(2):
                hsl = hloc[:, j * HID + f * P: j * HID + (f + 1) * P]
                nc.tensor.matmul(out=s1[f][:], lhsT=hsl, rhs=onescolb[:],
                                 start=(j == 0), stop=(j == CH - 1))
                nc.tensor.matmul(out=s2[f][:], lhsT=sq[:, f * P:(f + 1) * P],
                                 rhs=onescolb[:], start=(j == 0), stop=(j == CH - 1))
        st = pS.tile([P, 4], f32, tag="st")
        for f in range(2):
            nc.vector.tensor_copy(out=st[:, f:f + 1], in_=s1[f][:])
            nc.vector.tensor_copy(out=st[:, 2 + f:3 + f], in_=s2[f][:])
        dma(out=st1_in[:, :], in_=st[:])
    nc.gpsimd.collective_compute(
        "AllReduce", mybir.AluOpType.add, replica_groups=GRP,
        ins=[st1_in.ap().opt()], outs=[st1_out.ap().opt()])
    dma(out=stats1[:], in_=st1_out[:, :])

    def bn_fold(stats_sums, stats_sqs, gc, bec, scale_t, shift_t, pool):
        """stats cols -> scale/shift [P,2] (feature-column layout)."""
        mu = pool.tile([P, 2], f32, tag="mu")
        var = pool.tile([P, 2], f32, tag="var")
        tmp = pool.tile([P, 2], f32, tag="tmp")
        nc.vector.tensor_scalar_mul(out=mu[:], in0=stats_sums, scalar1=1.0 / N)
        nc.vector.tensor_scalar_mul(out=var[:], in0=stats_sqs, scalar1=1.0 / N)
        nc.vector.tensor_tensor(out=tmp[:], in0=mu[:], in1=mu[:], op=OP.mult)
        nc.vector.tensor_tensor(out=var[:], in0=var[:], in1=tmp[:], op=OP.subtract)
        nc.scalar.activation(out=tmp[:], in_=var[:], func=AF.Sqrt, bias=epscol[:])
        nc.vector.reciprocal(out=tmp[:], in_=tmp[:])
        nc.vector.tensor_tensor(out=scale_t[:], in0=gc[:], in1=tmp[:], op=OP.mult)
        nc.vector.tensor_tensor(out=tmp[:], in0=mu[:], in1=scale_t[:], op=OP.mult)
        nc.vector.tensor_tensor(out=shift_t[:], in0=bec[:], in1=tmp[:], op=OP.subtract)

    with tc.tile_pool(name="pB", bufs=1) as pB, \
         tc.tile_pool(name="pBp", bufs=1, space="PSUM") as pBp:
        bn_fold(stats1[:, 0:2], stats1[:, 2:4], g1c, be1c, scale1, shift1, pB)
        # W3' = scale1 (*) W3 rows; rrow = shift1 @ W3
        psr_ = pBp.tile([1, HID], f32, tag="rr")
        for f in range(2):
            w3f = pB.tile([P, HID], f32, tag="w3f")
            nc.vector.tensor_scalar_mul(out=w3f[:], in0=W3s[f][:],
                                        scalar1=scale1[:, f:f + 1])
            nc.vector.tensor_copy(out=W3p[f][:], in_=w3f[:])
            nc.tensor.matmul(out=psr_[:], lhsT=shift1[:, f:f + 1], rhs=W3s[f][:],
                             start=(f == 0), stop=(f == 1))
        nc.vector.tensor_copy(out=rrow[:], in_=psr_[:])
        nc.vector.tensor_copy(out=rrowb[:], in_=rrow[:])

    if STOP == 'B1':
        _early_out('B1'); ctx.close(); return

    # ============== hs3 rows = dis * (bn1(h2) @ W3), allgather ==============
    with tc.tile_pool(name="pH3", bufs=3, space="PSUM") as pH3:
        for j in range(CH):
            ps = pH3.tile([P, HID], f32, tag="hs3")
            nc.tensor.matmul(out=ps[:], lhsT=hT0[:, j * P:(j + 1) * P],
                             rhs=W3p[0][:], start=True, stop=False)
            nc.tensor.matmul(out=ps[:], lhsT=hT1[:, j * P:(j + 1) * P],
                             rhs=W3p[1][:], start=False, stop=False)
            nc.tensor.matmul(out=ps[:], lhsT=onesrowb[:], rhs=rrowb[:],
                             start=False, stop=True)
            nc.scalar.activation(out=hsloc[:, j * HID:(j + 1) * HID], in_=ps[:],
                                 func=AF.Copy, scale=disloc[:, j:j + 1])
            if j % 7 == 6:
                store_rows(hsloc3_d, j - 6, 7, hsloc)
    nc.gpsimd.collective_compute(
        "AllGather", mybir.AluOpType.bypass, replica_groups=GRP,
        ins=[hsloc3_d.ap().opt()], outs=[hs3_d.ap().opt()])
    if STOP == 'H3':
        _early_out('H3'); ctx.close(); return

    edge_pass(hs3_d, b3, do_hT=False, pfx="p3")
    if STOP == 'E3':
        _early_out('E3'); ctx.close(); return

    # ====== BN2 stats + pooled sums -> one allreduce ======
    with tc.tile_pool(name="pG2", bufs=4) as pG2, \
         tc.tile_pool(name="pGp", bufs=1, space="PSUM") as pGp:
        pool_ps = [pGp.tile([P, G], f32, tag=f"pool_{f}", name=f"pool_{f}") for f in range(2)]
        s1 = [pGp.tile([P, 1], f32, tag=f"gs1_{f}", name=f"gs1_{f}") for f in range(2)]
        s2 = [pGp.tile([P, 1], f32, tag=f"gs2_{f}", name=f"gs2_{f}") for f in range(2)]
        for j in range(CH):
            sq = pG2.tile([P, HID], bf16, tag="sq")
            nc.scalar.activation(out=sq[:], in_=hchunk(j), func=AF.Square)
            ohp = pG2.tile([P, G], bf16, tag="ohp")
            nc.vector.tensor_tensor(
                out=ohp[:], in0=batchloc[:, j:j + 1].to_broadcast([P, G]),
                in1=iota[:, 0:G], op=OP.is_equal)
            for f in range(2):
                hsl = hloc[:, j * HID + f * P: j * HID + (f + 1) * P]
                nc.tensor.matmul(out=pool_ps[f][:], lhsT=hsl, rhs=ohp[:],
                                 start=(j == 0), stop=(j == CH - 1))
                nc.tensor.matmul(out=s1[f][:], lhsT=hsl, rhs=onescolb[:],
                                 start=(j == 0), stop=(j == CH - 1))
                nc.tensor.matmul(out=s2[f][:], lhsT=sq[:, f * P:(f + 1) * P],
                                 rhs=onescolb[:], start=(j == 0), stop=(j == CH - 1))
        arp = pG2.tile([P, 132], f32, tag="arp")
        for f in range(2):
            nc.vector.tensor_copy(out=arp[:, f * G:(f + 1) * G], in_=pool_ps[f][:])
            nc.vector.tensor_copy(out=arp[:, 128 + f:129 + f], in_=s1[f][:])
            nc.vector.tensor_copy(out=arp[:, 130 + f:131 + f], in_=s2[f][:])
        dma(out=ar2_in[:, :], in_=arp[:])
    nc.gpsimd.collective_compute(
        "AllReduce", mybir.AluOpType.add, replica_groups=GRP,
        ins=[ar2_in.ap().opt()], outs=[ar2_out.ap().opt()])
    dma(out=ar2[:], in_=ar2_out[:, :])

    # ====== fold BN2 into Wf, final matmul ======
    with tc.tile_pool(name="pF", bufs=1) as pF, \
         tc.tile_pool(name="pFp", bufs=1, space="PSUM") as pFp:
        bn_fold(ar2[:, 128:130], ar2[:, 130:132], g2c, be2c, scale2, shift2, pF)
        psb = pFp.tile([1, NCLS], f32, tag="psb")
        for f in range(2):
            nc.vector.tensor_scalar_mul(out=Wfp[f][:], in0=Wfs[f][:],
                                        scalar1=scale2[:, f:f + 1])
            nc.tensor.matmul(out=psb[:], lhsT=shift2[:, f:f + 1], rhs=Wfs[f][:],
                             start=(f == 0), stop=False)
        nc.tensor.matmul(out=psb[:], lhsT=ones11[:], rhs=bfr[:],
                         start=False, stop=True)
        nc.vector.tensor_copy(out=bfp[:], in_=psb[:])

        pso = pFp.tile([G, NCLS], f32, tag="pso")
        for f in range(2):
            nc.tensor.matmul(out=pso[:], lhsT=ar2[:, f * G:(f + 1) * G],
                             rhs=Wfp[f][:], start=(f == 0), stop=False)
        nc.tensor.matmul(out=pso[:], lhsT=cntrow[:], rhs=bfp[:],
                         start=False, stop=True)
        osb = pF.tile([G, NCLS], f32, tag="osb")
        nc.vector.tensor_scalar_mul(out=osb[:], in0=pso[:], scalar1=invcnt[:])
        dma(out=out_d[:, :], in_=osb[:])

    ctx.close()


def kernel(x, edge_index, batch, W1, b1, W2, b2, W3, b3, g1, be1, g2, be2, Wf, bf):
    global LAST_EXEC_NS, LAST_RESULTS
    from concourse import bacc, tile
    from concourse.bass_utils import run_bass_kernel_spmd

    x = np.asarray(x)
    edge_index = np.asarray(edge_index)
    batch = np.asarray(batch)

    per_core, shared, T_half = _preprocess(x, edge_index, batch)

    nc = bacc.Bacc("TRN2", target_bir_lowering=False, debug=False,
                   num_devices=CORES)
    with tile.TileContext(nc) as tc:
        _build(nc, tc, T_half)
    nc.compile()

    def col2(v):
        return np.ascontiguousarray(np.asarray(v, F32).reshape(2, P).T)

    base = dict(
        xs=shared["xs"],
        iota128=shared["iota128"], eye128=shared["eye128"],
        onescolb=shared["onescolb"], onesrowb=shared["onesrowb"],
        ones11=shared["ones11"],
        cntrow=shared["cntrow"], invcntcol=shared["invcntcol"],
        W1=np.asarray(W1, F32).astype(BF16),
        W2=np.asarray(W2, F32).astype(BF16),
        W3=np.asarray(W3, F32),
        Wf=np.asarray(Wf, F32),
        b1=np.asarray(b1, F32).reshape(1, HID).astype(BF16),
        b2=np.asarray(b2, F32).reshape(1, HID).astype(BF16),
        b3=np.asarray(b3, F32).reshape(1, HID).astype(BF16),
        bf=np.asarray(bf, F32).reshape(1, NCLS),
        g1c=col2(g1), be1c=col2(be1), g2c=col2(g2), be2c=col2(be2),
    )
    in_maps = []
    for c in range(CORES):
        m = dict(base)
        m.update(per_core[c])
        in_maps.append(m)

    res = run_bass_kernel_spmd(nc, in_maps, core_ids=list(range(CORES)))
    LAST_EXEC_NS = res.exec_time_ns
    LAST_RESULTS = res
    return np.asarray(res.results[0]["out"], F32)


# revision 10
# speedup vs baseline: 3.6046x; 2.4745x over previous
"""GCN model (3x GCNConv + 2x BatchNorm + global mean pool + linear) on 8 TRN2 cores.

v3 strategy (vs v1 baseline):
- Host: bin-pack nodes into 392 bins of <=128 slots balancing per-bin edge
  counts (self-loops removed); remap node ids to (bin, slot); 49 bins per core
  (edges partitioned by dst bin).  Edges of each bin are split by src-table
  half (dma_gather indices are int16), padded per half to T_half tiles of 128.
  Upload a dis-prescaled x table [S, 128] bf16 as the layer-1 gather table
  (A(XW1) == (AX)W1, so W1 is applied after aggregation).
- Device (SPMD x8): batched dma_gather (SWDGE) fetches all source rows for a
  GROUP of bins in one instruction per table half (amortizes the ~1us fixed
  SWDGE cost that dominated v1's per-128-row indirect DMAs).  Per dst bin:
  one-hot (iota vs per-edge dst scalar, bf16 tensor_scalar 4x DVE mode)
  matmuls accumulate into PSUM; self-loop contribution via an identity matmul
  on the resident local table block (no gather); conv bias via K=1 bf16 matmul
  of invdis x b; relu + deg^-1/2 post-scale on the PSUM->SBUF copy.  Layer 1
  aggregates in [feat, dst] form so agg@W1 needs no transpose; layers 2/3 in
  [dst, feat] form with per-bin PE transposes into hT for the next layer's
  table matmuls.  Next-layer table rows stay resident in SBUF (reused for
  self-loops) and are stored via 7-bin batched DMAs.
- BatchNorms folded into following matmul weights (scale) + rank-1 shift.
- Cross-core: bf16 AllGather of node-feature tables (Shared scratchpad
  outputs) between layers; small AllReduces for BN stats / pooled sums.
"""

import os

import numpy as np
import ml_dtypes

N = 50000
E = 800000
IN = 128
HID = 256
G = 64
NCLS = 10
BN_EPS = 1e-5

P = 128
CORES = 8
BINS = 392          # global 128-slot bins (392*128 = 50176 slots)
CH = BINS // CORES  # 49 bins per core
SL = CH * P         # 6272 slots per core
S = BINS * P        # 50176 total slots
SHALF = S // 2      # dma_gather idx is int16: split table into two halves

F32 = np.float32
BF16 = ml_dtypes.bfloat16
I16 = np.int16

LAST_EXEC_NS = None
LAST_RESULTS = None


def _preprocess(x, edge_index, batch):
    import heapq

    src = edge_index[0].astype(np.int64)
    dst = edge_index[1].astype(np.int64)
    deg = 1.0 + np.bincount(dst, minlength=N).astype(np.float64)
    dis = (1.0 / np.sqrt(deg)).astype(F32)
    invdis = np.sqrt(deg).astype(F32)

    # ---- bin-pack nodes into BINS bins (<=128 nodes each), balancing edges ----
    w = np.bincount(dst, minlength=N).astype(np.int64)  # no self-loops
    order = np.argsort(-w, kind="stable")
    heap = [(0, b) for b in range(BINS)]
    heapq.heapify(heap)
    count = np.zeros(BINS, np.int64)
    new_id = np.empty(N, np.int64)
    for n in order:
        while True:
            load, b = heapq.heappop(heap)
            if count[b] < P:
                break
        new_id[n] = b * P + count[b]
        count[b] += 1
        heapq.heappush(heap, (load + int(w[n]), b))

    # ---- edges (NO self-loops), grouped by (dst bin, src half) ----
    es_s = new_id[src]
    ed_s = new_id[dst]
    bin_e = ed_s // P
    half_e = (es_s >= SHALF).astype(np.int64)
    key = bin_e * 2 + half_e
    o = np.argsort(key, kind="stable")
    es_s, ed_s, key = es_s[o], ed_s[o], key[o]
    half_s = key % 2
    cnt = np.bincount(key, minlength=BINS * 2)
    T_half = int(np.ceil(cnt.max() / P))
    cap = T_half * P
    starts = np.concatenate([[0], np.cumsum(cnt)[:-1]])
    rank = np.arange(len(es_s)) - starts[key]
    pos = key * cap + rank
    idx_pad = np.zeros(BINS * 2 * cap, np.int64)
    dst_pad = np.full(BINS * 2 * cap, 255.0, F32)
    idx_pad[pos] = es_s - half_s * SHALF          # row index within half table
    assert idx_pad.min() >= 0 and idx_pad.max() < SHALF
    dst_pad[pos] = (ed_s % P).astype(F32)
    # [bin, half, T_half, P]
    idx_pad = idx_pad.reshape(BINS, 2, T_half, P)
    dst_pad = dst_pad.reshape(BINS, 2, T_half, P)
    TT = 2 * T_half

    # ---- per-slot arrays ----
    slot_dis = np.zeros(S, F32)
    slot_dis[new_id] = dis
    slot_invdis = np.zeros(S, F32)
    slot_invdis[new_id] = invdis
    slot_batch = np.full(S, 255.0, F32)
    slot_batch[new_id] = batch.astype(F32)

    # layer-1 gather table: dis-prescaled x rows, [S, IN] bf16
    xs = np.zeros((S, IN), F32)
    xs[new_id, :] = x * dis[:, None]
    xs = xs.astype(BF16)
    xs_resh = xs.reshape(BINS, P, IN)  # [bin, slot, feat]

    cnts = np.bincount(batch.astype(np.int64), minlength=G).astype(F32)
    cnts = np.maximum(cnts, 1.0)

    def wrap16(lin):
        """Linear int array -> [128, len/16] int16, wrapped in 16 partitions."""
        w = lin.reshape(-1, 16).T.astype(I16)     # [16, len/16]
        return np.ascontiguousarray(np.tile(w, (8, 1)))

    per_core = []
    for c in range(CORES):
        b0, b1 = c * CH, (c + 1) * CH
        s0, s1 = c * SL, (c + 1) * SL
        xsl = np.transpose(xs_resh[b0:b1], (1, 0, 2))  # [P, CH, IN]
        # dstloc: [128, CH*TT]; bin-major, lo tiles then hi tiles
        dstl = dst_pad[b0:b1].reshape(CH * TT, P).T
        # gather index streams (bin-major within each half), linear order
        # (tile, lane):  [CH, T_half, P] -> flat
        idxlo = idx_pad[b0:b1, 0].reshape(-1)
        idxhi = idx_pad[b0:b1, 1].reshape(-1)
        per_core.append(
            dict(
                idxlo16=wrap16(idxlo),                                 # [128, CH*T_half*8] i16
                idxhi16=wrap16(idxhi),
                dstloc=np.ascontiguousarray(dstl).astype(BF16),         # [128, CH*TT]
                disloc=np.ascontiguousarray(slot_dis[s0:s1].reshape(CH, P).T),   # [128, CH]
                invdis=np.ascontiguousarray(
                    slot_invdis[s0:s1].reshape(1, SL)).astype(BF16),   # [1, SL]
                batchloc=np.ascontiguousarray(
                    slot_batch[s0:s1].reshape(CH, P).T).astype(BF16),  # [128, CH]
                xsloc=np.ascontiguousarray(xsl.reshape(P, CH * IN)),   # [128, CH*IN] bf16
            )
        )

    iota = np.tile(np.arange(P, dtype=F32), (P, 1))
    shared = dict(
        xs=xs,                                                     # [S, IN] bf16
        iota128=iota.astype(BF16),                                 # [128,128]
        eye128=np.eye(P, dtype=F32).astype(BF16),                  # [128,128]
        onescolb=np.ones((P, 1), F32).astype(BF16),
        onesrowb=np.ones((1, P), F32).astype(BF16),
        ones11=np.ones((1, 1), F32),
        cntrow=cnts.reshape(1, G),
        invcntcol=(1.0 / cnts).reshape(G, 1),
    )
    return per_core, shared, T_half


def _build(nc, tc, T_half):
    from concourse import bass, mybir
    STOP = os.environ.get('K_STOP', '')
    SHARED = os.environ.get('K_LOCAL_TABLES', '') != '1'

    f32 = mybir.dt.float32
    bf16 = mybir.dt.bfloat16
    i16 = mybir.dt.int16
    AF = mybir.ActivationFunctionType
    OP = mybir.AluOpType
    TT = 2 * T_half
    NI16 = CH * T_half * 8   # idx16 cols per half

    # ---------------- parameters ----------------
    def par(name, shape, dt):
        return nc.declare_dram_parameter(name, list(shape), dt, isOutput=False)

    xs_d = par("xs", (S, IN), bf16)
    idxlo_d = par("idxlo16", (P, NI16), i16)
    idxhi_d = par("idxhi16", (P, NI16), i16)
    dstloc_d = par("dstloc", (P, CH * TT), bf16)
    disloc_d = par("disloc", (P, CH), f32)
    invdis_d = par("invdis", (1, SL), bf16)
    batch_d = par("batchloc", (P, CH), bf16)
    xsloc_d = par("xsloc", (P, CH * IN), bf16)
    iota_d = par("iota128", (P, P), bf16)
    eye_d = par("eye128", (P, P), bf16)
    onescolb_d = par("onescolb", (P, 1), bf16)
    onesrowb_d = par("onesrowb", (1, P), bf16)
    ones11_d = par("ones11", (1, 1), f32)
    cntrow_d = par("cntrow", (1, G), f32)
    invcnt_d = par("invcntcol", (G, 1), f32)
    W1_d = par("W1", (IN, HID), bf16)
    W2_d = par("W2", (HID, HID), bf16)
    W3_d = par("W3", (HID, HID), f32)
    Wf_d = par("Wf", (HID, NCLS), f32)
    b1_d = par("b1", (1, HID), bf16)
    b2_d = par("b2", (1, HID), bf16)
    b3_d = par("b3", (1, HID), bf16)
    bf_d = par("bf", (1, NCLS), f32)
    g1_d = par("g1c", (P, 2), f32)
    be1_d = par("be1c", (P, 2), f32)
    g2_d = par("g2c", (P, 2), f32)
    be2_d = par("be2c", (P, 2), f32)
    out_d = nc.declare_dram_parameter("out", [G, NCLS], f32, isOutput=True)

    # ---------------- device DRAM ----------------
    addr = dict(addr_space="Shared") if SHARED else {}
    hsloc2_d = nc.dram_tensor("hsloc2", [SL, HID], bf16)
    hsloc3_d = nc.dram_tensor("hsloc3", [SL, HID], bf16)
    hs2_d = nc.dram_tensor("hs2", [S, HID], bf16, **addr)
    hs3_d = nc.dram_tensor("hs3", [S, HID], bf16, **addr)
    st1_in = nc.dram_tensor("st1_in", [P, 4], f32)
    st1_out = nc.dram_tensor("st1_out", [P, 4], f32)
    ar2_in = nc.dram_tensor("ar2_in", [P, 132], f32)
    ar2_out = nc.dram_tensor("ar2_out", [P, 132], f32)

    GRP = [list(range(CORES))]

    # ---------------- resident SBUF ----------------
    import contextlib

    ctx = contextlib.ExitStack()
    res = ctx.enter_context(tc.tile_pool(name="res", bufs=1))

    idxlo = res.tile([P, NI16], i16)
    idxhi = res.tile([P, NI16], i16)
    dstloc = res.tile([P, CH * TT], bf16)
    disloc = res.tile([P, CH], f32)
    invdis = res.tile([1, SL], bf16)
    batchloc = res.tile([P, CH], bf16)
    xsloc = res.tile([P, CH * IN], bf16)
    hloc = res.tile([P, CH * HID], bf16)    # conv output h per local bin
    hT0 = res.tile([P, SL], bf16)           # h transposed, feat block 0
    hT1 = res.tile([P, SL], bf16)
    hsloc = res.tile([P, CH * HID], bf16)   # next-layer table rows (local bins)
    iota = res.tile([P, P], bf16)
    eye = res.tile([P, P], bf16)
    onescolb = res.tile([P, 1], bf16)
    onesrowb = res.tile([1, P], bf16)
    ones11 = res.tile([1, 1], f32)
    cntrow = res.tile([1, G], f32)
    invcnt = res.tile([G, 1], f32)
    W1 = res.tile([IN, HID], bf16)
    W2s = [res.tile([P, HID], bf16, tag=f"w2_{f}", name=f"w2_{f}") for f in range(2)]
    W3s = [res.tile([P, HID], f32, tag=f"w3_{f}", name=f"w3_{f}") for f in range(2)]
    W3p = [res.tile([P, HID], bf16, tag=f"w3p_{f}", name=f"w3p_{f}") for f in range(2)]
    Wfs = [res.tile([P, NCLS], f32, tag=f"wf_{f}", name=f"wf_{f}") for f in range(2)]
    Wfp = [res.tile([P, NCLS], f32, tag=f"wfp_{f}", name=f"wfp_{f}") for f in range(2)]
    b1 = res.tile([1, HID], bf16)
    b2 = res.tile([1, HID], bf16)
    b3 = res.tile([1, HID], bf16)
    bfr = res.tile([1, NCLS], f32)
    g1c = res.tile([P, 2], f32)
    be1c = res.tile([P, 2], f32)
    g2c = res.tile([P, 2], f32)
    be2c = res.tile([P, 2], f32)
    rrow = res.tile([1, HID], f32)
    rrowb = res.tile([1, HID], bf16)
    bfp = res.tile([1, NCLS], f32)
    scale1 = res.tile([P, 2], f32)
    shift1 = res.tile([P, 2], f32)
    scale2 = res.tile([P, 2], f32)
    shift2 = res.tile([P, 2], f32)
    stats1 = res.tile([P, 4], f32)
    ar2 = res.tile([P, 132], f32)
    epscol = res.tile([P, 1], f32)
    nc.vector.memset(epscol[:], BN_EPS)

    dma = nc.sync.dma_start
    for dst_t, src_t in [
        (idxlo, idxlo_d), (idxhi, idxhi_d), (dstloc, dstloc_d),
        (disloc, disloc_d), (invdis, invdis_d), (batchloc, batch_d),
        (xsloc, xsloc_d), (iota, iota_d), (eye, eye_d),
        (onescolb, onescolb_d), (onesrowb, onesrowb_d), (ones11, ones11_d),
        (cntrow, cntrow_d), (invcnt, invcnt_d), (W1, W1_d),
        (b1, b1_d), (b2, b2_d), (b3, b3_d), (bfr, bf_d),
        (g1c, g1_d), (be1c, be1_d), (g2c, g2_d), (be2c, be2_d),
    ]:
        dma(out=dst_t[:], in_=src_t[:, :])
    for f in range(2):
        dma(out=W2s[f][:], in_=W2_d[f * P:(f + 1) * P, :])
        dma(out=W3s[f][:], in_=W3_d[f * P:(f + 1) * P, :])
        dma(out=Wfs[f][:], in_=Wf_d[f * P:(f + 1) * P, :])

    # ---------------- AP helpers ----------------
    def ap3(tile2d, col0, k, inner):
        """View tile2d[:, col0:col0+k*inner] as [128, k, inner]."""
        a = tile2d[:, col0:col0 + k * inner]
        return bass.AP(a.tensor, a.offset, [list(a.ap[0]), [inner, k], [1, inner]])

    def store_rows(dram_t, j0, nb, sb_tile):
        """DMA sb_tile[:, j0*HID:(j0+nb)*HID] -> dram rows [j0*128,(j0+nb)*128)."""
        full = dram_t.ap()
        d_ap = bass.AP(full.tensor, j0 * P * HID,
                       [[HID, P], [P * HID, nb], [1, HID]])
        s = sb_tile[:, j0 * HID:(j0 + nb) * HID]
        s_ap = bass.AP(s.tensor, s.offset, [list(s.ap[0]), [HID, nb], [1, HID]])
        dma(out=d_ap, in_=s_ap)

    def hchunk(j):
        return hloc[:, j * HID:(j + 1) * HID]

    CT = 8                                 # tiles per dma_gather (ring limit)
    NSTREAM = CH * T_half                  # tiles per half-stream per pass

    def make_stream(pool, table, elem, tag):
        """Streamed dma_gather over a pass's tiles: one call per CT tiles per
        half.  get(half, pos) -> SBUF AP of that tile; ensure(j) prefetches
        through the end of bin j."""
        tiles = {0: {}, 1: {}}
        halves = ((idxlo, table[0:SHALF, :]), (idxhi, table[SHALF:S, :]))
        qctr = [0]

        def _issue(h, c):
            if c in tiles[h] or c * CT >= NSTREAM:
                return
            n_t = min(CT, NSTREAM - c * CT)
            idx_t, rows = halves[h]
            gt = pool.tile([P, CT * elem], bf16, tag=f"{tag}{h}")
            n = n_t * P
            nc.gpsimd.dma_gather(
                ap3(gt, 0, n_t, elem), rows,
                idx_t[:, c * CT * 8:c * CT * 8 + n // 16],
                n, n, elem, queue_num=qctr[0] % 4,
            )
            qctr[0] += 1
            tiles[h][c] = gt

        def ensure(j):
            cmax = min((j + 1) * T_half + T_half - 1, NSTREAM - 1) // CT
            for h in (0, 1):
                c0 = max(tiles[h].keys(), default=-1) + 1
                for c in range(c0, cmax + 1):
                    _issue(h, c)

        def get(h, pos):
            c, s = divmod(pos, CT)
            return tiles[h][c][:, s * elem:(s + 1) * elem]

        return ensure, get

    def bcast_mid(tile2d, k, inner):
        a = tile2d[:, 0:inner]
        return bass.AP(a.tensor, a.offset, [list(a.ap[0]), [0, k], [1, inner]])

    def onehots(pool, j, tag):
        """All TT one-hot [edge, dst] tiles for bin j (bf16), 4 tiles/DVE op."""
        ohs = pool.tile([P, TT * P], bf16, tag=tag)
        b = 0
        while b < TT:
            k = min(4, TT - b)
            nc.vector.tensor_tensor(
                out=ap3(ohs, b * P, k, P),
                in0=dstloc[:, j * TT + b:j * TT + b + k].to_broadcast([P, k, P]),
                in1=bcast_mid(iota, k, P),
                op=OP.is_equal)
            b += k
        return ohs

    def transpose_to_hT(pT, j):
        for f, hT in enumerate((hT0, hT1)):
            pt = pT.tile([P, P], bf16, tag=f"pt{f}")
            nc.tensor.transpose(
                out=pt[:], in_=hloc[:, j * HID + f * P: j * HID + (f + 1) * P],
                identity=eye[:])
            nc.vector.tensor_copy(out=hT[:, j * P:(j + 1) * P], in_=pt[:])

    def _early_out(tag):
        with tc.tile_pool(name="eo_" + tag, bufs=1) as eo:
            z = eo.tile([G, NCLS], f32, tag="z", name="z_" + tag)
            nc.vector.tensor_copy(out=z[:], in_=hloc[0:G, 0:NCLS])
            dma(out=out_d[:, :], in_=z[:])

    # ============ E1: layer-1 aggregation from xs + h1 + hs2 rows ============
    with tc.tile_pool(name="pG1", bufs=4) as pG1, \
         tc.tile_pool(name="pOh1", bufs=2) as pOh1, \
         tc.tile_pool(name="pA1", bufs=2) as pA1, \
         tc.tile_pool(name="pPs1", bufs=2, space="PSUM") as pPs1, \
         tc.tile_pool(name="pH1", bufs=2, space="PSUM") as pH1, \
         tc.tile_pool(name="pT1", bufs=1, space="PSUM") as pT1:
        ens1, get1 = make_stream(pG1, xs_d, IN, "g1")
        for j in range(CH):
            ens1(j)
            if True:
                ohs = onehots(pOh1, j, "oh")
                # aggT[feat, dst] in PSUM: self-loop (identity) + TT edge tiles
                ps1 = pPs1.tile([P, P], f32, tag="agg")
                nc.tensor.matmul(out=ps1[:], lhsT=xsloc[:, j * IN:(j + 1) * IN],
                                 rhs=eye[:], start=True, stop=False)
                for t in range(TT):
                    h = 0 if t < T_half else 1
                    pos = j * T_half + (t if t < T_half else t - T_half)
                    nc.tensor.matmul(out=ps1[:], lhsT=get1(h, pos),
                                     rhs=ohs[:, t * P:(t + 1) * P],
                                     start=False, stop=(t == TT - 1))
                aT = pA1.tile([P, P], bf16, tag="aT")
                nc.vector.tensor_copy(out=aT[:], in_=ps1[:])
                # h1 = relu(dis * (agg @ W1) + b1)
                ps2 = pH1.tile([P, HID], f32, tag="h1")
                nc.tensor.matmul(out=ps2[:], lhsT=aT[:], rhs=W1[:],
                                 start=True, stop=False)
                nc.tensor.matmul(out=ps2[:], lhsT=invdis[0:1, j * P:(j + 1) * P],
                                 rhs=b1[:], start=False, stop=True)
                nc.scalar.activation(out=hchunk(j), in_=ps2[:], func=AF.Relu,
                                     scale=disloc[:, j:j + 1])
                transpose_to_hT(pT1, j)
                # hs2 row = dis * (h1 @ W2)
                ps3 = pH1.tile([P, HID], f32, tag="hs2")
                nc.tensor.matmul(out=ps3[:], lhsT=hT0[:, j * P:(j + 1) * P],
                                 rhs=W2s[0][:], start=True, stop=False)
                nc.tensor.matmul(out=ps3[:], lhsT=hT1[:, j * P:(j + 1) * P],
                                 rhs=W2s[1][:], start=False, stop=True)
                nc.scalar.activation(out=hsloc[:, j * HID:(j + 1) * HID], in_=ps3[:],
                                     func=AF.Copy, scale=disloc[:, j:j + 1])
                if j % 7 == 6:
                    store_rows(hsloc2_d, j - 6, 7, hsloc)
    nc.gpsimd.collective_compute(
        "AllGather", mybir.AluOpType.bypass, replica_groups=GRP,
        ins=[hsloc2_d.ap().opt()], outs=[hs2_d.ap().opt()])

    if STOP == 'E1':
        _early_out('E1'); ctx.close(); return

    # ============ E2/E3: layers 2/3 aggregation from hs tables ============
    def edge_pass(hs_table, bias_row, do_hT, pfx):
        with tc.tile_pool(name=pfx + "G", bufs=4) as pG, \
             tc.tile_pool(name=pfx + "Oh", bufs=2) as pOh, \
             tc.tile_pool(name=pfx + "Ps", bufs=2, space="PSUM") as pPs, \
             tc.tile_pool(name=pfx + "T", bufs=2, space="PSUM") as pT:
            ens, get = make_stream(pG, hs_table, HID, "g")
            for j in range(CH):
                ens(j)
                if True:
                    ohs = onehots(pOh, j, "oh")
                    ps = pPs.tile([P, HID], f32, tag="agg")
                    for t in range(TT):
                        h = 0 if t < T_half else 1
                        pos = j * T_half + (t if t < T_half else t - T_half)
                        nc.tensor.matmul(out=ps[:], lhsT=ohs[:, t * P:(t + 1) * P],
                                         rhs=get(h, pos),
                                         start=(t == 0), stop=False)
                    # self-loop: psum[dst,:] += hsloc_bin[dst,:]
                    nc.tensor.matmul(out=ps[:], lhsT=eye[:],
                                     rhs=hsloc[:, j * HID:(j + 1) * HID],
                                     start=False, stop=False)
                    # + (1/dis) x bias (K=1), then relu(dis * psum)
                    nc.tensor.matmul(out=ps[:], lhsT=invdis[0:1, j * P:(j + 1) * P],
                                     rhs=bias_row[:], start=False, stop=True)
                    nc.scalar.activation(out=hchunk(j), in_=ps[:], func=AF.Relu,
                                         scale=disloc[:, j:j + 1])
                    if do_hT:
                        transpose_to_hT(pT, j)

    edge_pass(hs2_d, b2, do_hT=True, pfx="p2")
    if STOP == 'E2':
        _early_out('E2'); ctx.close(); return

    # ============== BN1 stats -> allreduce -> scale1/shift1 ==============
    with tc.tile_pool(name="pS", bufs=4) as pS, \
         tc.tile_pool(name="pSp", bufs=1, space="PSUM") as pSp:
        s1 = [pSp.tile([P, 1], f32, tag=f"s1_{f}", name=f"s1_{f}") for f in range(2)]
        s2 = [pSp.tile([P, 1], f32, tag=f"s2_{f}", name=f"s2_{f}") for f in range(2)]
        for j in range(CH):
            sq = pS.tile([P, HID], bf16, tag="sq")
            nc.scalar.activation(out=sq[:], in_=hchunk(j), func=AF.Square)
            for f in range(2):
                hsl = hloc[:, j * HID + f * P: j * HID + (f + 1) * P]
                nc.tensor.matmul(out=s1[f][:], lhsT=hsl, rhs=onescolb[:],
                                 start=(j == 0), stop=(j == CH - 1))
                nc.tensor.matmul(out=s2[f][:], lhsT=sq[:, f * P:(f + 1) * P],
                                 rhs=onescolb[:], start=(j == 0), stop=(j == CH - 1))
        st = pS.tile([P, 4], f32, tag="st")
        for f in range(2):
            nc.vector.tensor_copy(out=st[:, f:f + 1], in_=s1[f][:])
            nc.vector.tensor_copy(out=st[:, 2 + f:3 + f], in_=s2[f][:])
        dma(out=st1_in[:, :], in_=st[:])
    nc.gpsimd.collective_compute(
        "AllReduce", mybir.AluOpType.add, replica_groups=GRP,
        ins=[st1_in.ap().opt()], outs=[st1_out.ap().opt()])
    dma(out=stats1[:], in_=st1_out[:, :])

    def bn_fold(stats_sums, stats_sqs, gc, bec, scale_t, shift_t, pool):
        """stats cols -> scale/shift [P,2] (feature-column layout)."""
        mu = pool.tile([P, 2], f32, tag="mu")
        var = pool.tile([P, 2], f32, tag="var")
        tmp = pool.tile([P, 2], f32, tag="tmp")
        nc.vector.tensor_scalar_mul(out=mu[:], in0=stats_sums, scalar1=1.0 / N)
        nc.vector.tensor_scalar_mul(out=var[:], in0=stats_sqs, scalar1=1.0 / N)
        nc.vector.tensor_tensor(out=tmp[:], in0=mu[:], in1=mu[:], op=OP.mult)
        nc.vector.tensor_tensor(out=var[:], in0=var[:], in1=tmp[:], op=OP.subtract)
        nc.scalar.activation(out=tmp[:], in_=var[:], func=AF.Sqrt, bias=epscol[:])
        nc.vector.reciprocal(out=tmp[:], in_=tmp[:])
        nc.vector.tensor_tensor(out=scale_t[:], in0=gc[:], in1=tmp[:], op=OP.mult)
        nc.vector.tensor_tensor(out=tmp[:], in0=mu[:], in1=scale_t[:], op=OP.mult)
        nc.vector.tensor_tensor(out=shift_t[:], in0=bec[:], in1=tmp[:], op=OP.subtract)

    with tc.tile_pool(name="pB", bufs=1) as pB, \
         tc.tile_pool(name="pBp", bufs=1, space="PSUM") as pBp:
        bn_fold(stats1[:, 0:2], stats1[:, 2:4], g1c, be1c, scale1, shift1, pB)
        # W3' = scale1 (*) W3 rows; rrow = shift1 @ W3
        psr_ = pBp.tile([1, HID], f32, tag="rr")
        for f in range(2):
            w3f = pB.tile([P, HID], f32, tag="w3f")
            nc.vector.tensor_scalar_mul(out=w3f[:], in0=W3s[f][:],
                                        scalar1=scale1[:, f:f + 1])
            nc.vector.tensor_copy(out=W3p[f][:], in_=w3f[:])
            nc.tensor.matmul(out=psr_[:], lhsT=shift1[:, f:f + 1], rhs=W3s[f][:],
                             start=(f == 0), stop=(f == 1))
        nc.vector.tensor_copy(out=rrow[:], in_=psr_[:])
        nc.vector.tensor_copy(out=rrowb[:], in_=rrow[:])

    if STOP == 'B1':
        _early_out('B1'); ctx.close(); return

    # ============== hs3 rows = dis * (bn1(h2) @ W3), allgather ==============
    with tc.tile_pool(name="pH3", bufs=3, space="PSUM") as pH3:
        for j in range(CH):
            ps = pH3.tile([P, HID], f32, tag="hs3")
            nc.tensor.matmul(out=ps[:], lhsT=hT0[:, j * P:(j + 1) * P],
                             rhs=W3p[0][:], start=True, stop=False)
            nc.tensor.matmul(out=ps[:], lhsT=hT1[:, j * P:(j + 1) * P],
                             rhs=W3p[1][:], start=False, stop=False)
            nc.tensor.matmul(out=ps[:], lhsT=onesrowb[:], rhs=rrowb[:],
                             start=False, stop=True)
            nc.scalar.activation(out=hsloc[:, j * HID:(j + 1) * HID], in_=ps[:],
                                 func=AF.Copy, scale=disloc[:, j:j + 1])
            if j % 7 == 6:
                store_rows(hsloc3_d, j - 6, 7, hsloc)
    nc.gpsimd.collective_compute(
        "AllGather", mybir.AluOpType.bypass, replica_groups=GRP,
        ins=[hsloc3_d.ap().opt()], outs=[hs3_d.ap().opt()])
    if STOP == 'H3':
        _early_out('H3'); ctx.close(); return

    edge_pass(hs3_d, b3, do_hT=False, pfx="p3")
    if STOP == 'E3':
        _early_out('E3'); ctx.close(); return

    # ====== BN2 stats + pooled sums -> one allreduce ======
    with tc.tile_pool(name="pG2", bufs=4) as pG2, \
         tc.tile_pool(name="pGp", bufs=1, space="PSUM") as pGp:
        pool_ps = [pGp.tile([P, G], f32, tag=f"pool_{f}", name=f"pool_{f}") for f in range(2)]
        s1 = [pGp.tile([P, 1], f32, tag=f"gs1_{f}", name=f"gs1_{f}") for f in range(2)]
        s2 = [pGp.tile([P, 1], f32, tag=f"gs2_{f}", name=f"gs2_{f}") for f in range(2)]
        for j in range(CH):
            sq = pG2.tile([P, HID], bf16, tag="sq")
            nc.scalar.activation(out=sq[:], in_=hchunk(j), func=AF.Square)
            ohp = pG2.tile([P, G], bf16, tag="ohp")
            nc.vector.tensor_tensor(
                out=ohp[:], in0=batchloc[:, j:j + 1].to_broadcast([P, G]),
                in1=iota[:, 0:G], op=OP.is_equal)
            for f in range(2):
                hsl = hloc[:, j * HID + f * P: j * HID + (f + 1) * P]
                nc.tensor.matmul(out=pool_ps[f][:], lhsT=hsl, rhs=ohp[:],
                                 start=(j == 0), stop=(j == CH - 1))
                nc.tensor.matmul(out=s1[f][:], lhsT=hsl, rhs=onescolb[:],
                                 start=(j == 0), stop=(j == CH - 1))
                nc.tensor.matmul(out=s2[f][:], lhsT=sq[:, f * P:(f + 1) * P],
                                 rhs=onescolb[:], start=(j == 0), stop=(j == CH - 1))
        arp = pG2.tile([P, 132], f32, tag="arp")
        for f in range(2):
            nc.vector.tensor_copy(out=arp[:, f * G:(f + 1) * G], in_=pool_ps[f][:])
            nc.vector.tensor_copy(out=arp[:, 128 + f:129 + f], in_=s1[f][:])
            nc.vector.tensor_copy(out=arp[:, 130 + f:131 + f], in_=s2[f][:])
        dma(out=ar2_in[:, :], in_=arp[:])
    nc.gpsimd.collective_compute(
        "AllReduce", mybir.AluOpType.add, replica_groups=GRP,
        ins=[ar2_in.ap().opt()], outs=[ar2_out.ap().opt()])
    dma(out=ar2[:], in_=ar2_out[:, :])

    # ====== fold BN2 into Wf, final matmul ======
    with tc.tile_pool(name="pF", bufs=1) as pF, \
         tc.tile_pool(name="pFp", bufs=1, space="PSUM") as pFp:
        bn_fold(ar2[:, 128:130], ar2[:, 130:132], g2c, be2c, scale2, shift2, pF)
        psb = pFp.tile([1, NCLS], f32, tag="psb")
        for f in range(2):
            nc.vector.tensor_scalar_mul(out=Wfp[f][:], in0=Wfs[f][:],
                                        scalar1=scale2[:, f:f + 1])
            nc.tensor.matmul(out=psb[:], lhsT=shift2[:, f:f + 1], rhs=Wfs[f][:],
                             start=(f == 0), stop=False)
        nc.tensor.matmul(out=psb[:], lhsT=ones11[:], rhs=bfr[:],
                         start=False, stop=True)
        nc.vector.tensor_copy(out=bfp[:], in_=psb[:])

        pso = pFp.tile([G, NCLS], f32, tag="pso")
        for f in range(2):
            nc.tensor.matmul(out=pso[:], lhsT=ar2[:, f * G:(f + 1) * G],
                             rhs=Wfp[f][:], start=(f == 0), stop=False)
        nc.tensor.matmul(out=pso[:], lhsT=cntrow[:], rhs=bfp[:],
                         start=False, stop=True)
        osb = pF.tile([G, NCLS], f32, tag="osb")
        nc.vector.tensor_scalar_mul(out=osb[:], in0=pso[:], scalar1=invcnt[:])
        dma(out=out_d[:, :], in_=osb[:])

    ctx.close()


def kernel(x, edge_index, batch, W1, b1, W2, b2, W3, b3, g1, be1, g2, be2, Wf, bf):
    global LAST_EXEC_NS, LAST_RESULTS
    from concourse import bacc, tile
    from concourse.bass_utils import run_bass_kernel_spmd

    x = np.asarray(x)
    edge_index = np.asarray(edge_index)
    batch = np.asarray(batch)

    per_core, shared, T_half = _preprocess(x, edge_index, batch)

    nc = bacc.Bacc("TRN2", target_bir_lowering=False, debug=False,
                   num_devices=CORES, num_swdge_queues=4)
    with tile.TileContext(nc) as tc:
        _build(nc, tc, T_half)
    nc.compile()

    def col2(v):
        return np.ascontiguousarray(np.asarray(v, F32).reshape(2, P).T)

    base = dict(
        xs=shared["xs"],
        iota128=shared["iota128"], eye128=shared["eye128"],
        onescolb=shared["onescolb"], onesrowb=shared["onesrowb"],
        ones11=shared["ones11"],
        cntrow=shared["cntrow"], invcntcol=shared["invcntcol"],
        W1=np.asarray(W1, F32).astype(BF16),
        W2=np.asarray(W2, F32).astype(BF16),
        W3=np.asarray(W3, F32),
        Wf=np.asarray(Wf, F32),
        b1=np.asarray(b1, F32).reshape(1, HID).astype(BF16),
        b2=np.asarray(b2, F32).reshape(1, HID).astype(BF16),
        b3=np.asarray(b3, F32).reshape(1, HID).astype(BF16),
        bf=np.asarray(bf, F32).reshape(1, NCLS),
        g1c=col2(g1), be1c=col2(be1), g2c=col2(g2), be2c=col2(be2),
    )
    in_maps = []
    for c in range(CORES):
        m = dict(base)
        m.update(per_core[c])
        in_maps.append(m)

    res = run_bass_kernel_spmd(nc, in_maps, core_ids=list(range(CORES)))
    LAST_EXEC_NS = res.exec_time_ns
    LAST_RESULTS = res
    return np.asarray(res.results[0]["out"], F32)
